# revision 26
# baseline (speedup 1.0000x reference)
"""Trainium2 Bass kernel for nn_CNN_PHMM_VAE loss (profile-HMM forward + VAE KLD).

Data parallel over 8 NeuronCores (64 examples per core). PHMM forward in
probability space. No runtime rescaling: a global per-step growth factor
e^C_DECAY is folded into the host-side tables so the bf16 state stays in
range, and the exact inverse is a compile-time constant in the final log.
The NEG=-100 restart boundary (which dominates the tail of the sequence)
is carried exactly: its g^l growth lives in per-step table slots that are
multiplied against constant-1.0 lanes of the t tile.

Host tables (per step l, 65 wide):
  EE2[b,l,0]  = A1[0]*sig0*g^(l+1)        EE2[b,l,1+j] = g*A1[j+1]*e(j,s_l)
  EU2[b,l,0]  = A3[0]*sig0*g^(l+1)        EU2[b,l,1]   = U[1]*A1[0]*sig0*g^(l+1)
  EU2[b,l,2+j] = U[j+2]*EE2[b,l,1+j]
t tile layout: tpad = [1.0, 1.0, t[0:64]]; mu-mul reads tpad[1:66],
muU-mul reads tpad[0:65], so the boundary slots come from the tables.

Per step, 7 DVE ops ordered [scan, r12, t, ya, mu, muU, y'] so every
producer->consumer pair is >=2 apart (no adjacent-dependency stalls):
  delta = scan(V, MUU)        ; r12 = [G1-G2|G2] o X
  t     = y + delta           ; ya  = r12.lo + r12.hi
  mu'   = EE2_l o tpad        ; MUU = EU2_l o tpad
  y'    = mu' + ya
"""
import numpy as np

B, L, K, E = 512, 256, 64, 16
K1 = K + 1
N_CORES = 8
BPC = B // N_CORES
LOGACC0 = -25.0
NEG = -100.0
C_DECAY = 0.52           # per-step growth folded into tables
M2M, M2I, M2D, I2M, I2I, D2M, D2D = 0, 1, 2, 3, 4, 5, 6

# --- small f32 table layout -------------------------------------------------
OFF_X0 = 0             # 132: initial [mu | pad | y | pad]
OFF_V = 132            # 65
OFF_GG = 198           # 132: [G1-G2 | G2] aligned to X layout
OFF_MUU0 = 330         # 65: initial MUU (= beta column at l=0)
OFF_MUS = 400          # 16
OFF_LV = 416           # 16
TBL_W = 432

XW = 132   # state: mu 0..64, pad, y 66..130, pad
YO = 66

_CACHED = {}


def _host_tables(batch_input, transition_probs, emission_probs, mus, logvars):
    import ml_dtypes

    a = np.asarray(transition_probs, np.float64)
    Earr = np.exp(np.asarray(emission_probs, np.float64))
    s = np.asarray(batch_input)
    A1 = np.exp(a[:, :, M2M])
    A2 = np.exp(a[:, :, I2M])
    A3 = np.exp(a[:, :, D2M])
    B1 = 0.25 * np.exp(a[:, :, M2I])
    B2 = 0.25 * np.exp(a[:, :, I2I])
    C1 = np.exp(a[:, :, M2D])
    C2 = np.exp(a[:, :, D2D])

    U = np.zeros((B, K1)); V = np.zeros((B, K1))
    U[:, 1:] = A3[:, 1:] * C1[:, :-1] / A1[:, :-1]
    V[:, 1:] = A3[:, 1:] * C2[:, :-1] / A3[:, :-1]
    g = np.exp(C_DECAY)
    G1 = g * A2 * B1 / A1
    G2 = g * B2

    Etil = g * A1[:, 1:, None] * Earr
    ee = Etil[np.arange(B)[:, None, None], np.arange(K)[None, None, :],
              s[:, :, None]]                      # (B, L, K)
    sig0 = np.exp(NEG - LOGACC0)
    e0 = np.exp(-LOGACC0)
    gpow = g ** (np.arange(L) + 1.0)              # g^(l+1)
    EE2 = np.empty((B, L, K1))
    EU2 = np.empty((B, L, K1))
    EE2[:, :, 0] = A1[:, 0:1] * sig0 * gpow[None, :]
    EE2[:, :, 1:] = ee
    EU2[:, :, 0] = A3[:, 0:1] * sig0 * gpow[None, :]
    EU2[:, :, 1] = U[:, 1:2] * A1[:, 0:1] * sig0 * gpow[None, :]
    EU2[:, :, 2:] = U[:, None, 2:] * ee[:, :, :K - 1]
    ee_bf = np.asarray(EE2, ml_dtypes.bfloat16).reshape(B, L * K1)
    eeU_bf = np.asarray(EU2, ml_dtypes.bfloat16).reshape(B, L * K1)

    mu0 = np.empty((B, K1)); iot0 = np.empty((B, K1))
    mu0[:, 0] = A1[:, 0] * e0
    mu0[:, 1:] = A1[:, 1:] * sig0
    iot0[:, :] = A2 * sig0
    muu0 = np.empty((B, K1))
    muu0[:, 0] = A3[:, 0] * sig0
    muu0[:, 1:] = U[:, 1:] * mu0[:, :-1]

    tbl = np.zeros((B, TBL_W), np.float32)
    tbl[:, OFF_X0:OFF_X0 + K1] = mu0
    tbl[:, OFF_X0 + YO:OFF_X0 + YO + K1] = mu0 + iot0
    tbl[:, OFF_V:OFF_V + K1] = V
    tbl[:, OFF_GG:OFF_GG + K1] = G1 - G2
    tbl[:, OFF_GG + YO:OFF_GG + YO + K1] = G2
    tbl[:, OFF_MUU0:OFF_MUU0 + K1] = muu0
    tbl[:, OFF_MUS:OFF_MUS + E] = np.asarray(mus, np.float32)
    tbl[:, OFF_LV:OFF_LV + E] = np.asarray(logvars, np.float32)
    return tbl, ee_bf, eeU_bf


def _build_bass():
    import concourse.tile as tile
    from concourse import bacc, mybir
    from contextlib import ExitStack

    f32 = mybir.dt.float32
    bf = mybir.dt.bfloat16
    mult = mybir.AluOpType.mult
    add = mybir.AluOpType.add
    AF = mybir.ActivationFunctionType

    nc = bacc.Bacc("TRN2", target_bir_lowering=False, debug=False,
                   num_devices=N_CORES)
    tbl_d = nc.dram_tensor("tbl", [BPC, TBL_W], f32, kind="ExternalInput").ap()
    ee_d = nc.dram_tensor("ee", [BPC, L * K1], bf, kind="ExternalInput").ap()
    eeU_d = nc.dram_tensor("eeU", [BPC, L * K1], bf,
                           kind="ExternalInput").ap()
    out_d = nc.dram_tensor("loss", [BPC, 1], f32, kind="ExternalOutput").ap()

    with tile.TileContext(nc) as tc, ExitStack() as ctx:
        ctx.enter_context(nc.allow_low_precision(
            reason="bf16 DP state validated to ~2.4e-4 per-example on the loss"))
        pool = ctx.enter_context(tc.tile_pool(name="p", bufs=1))

        TBL = pool.tile([BPC, TBL_W], f32, tag="TBL", name="TBL")
        EEt = pool.tile([BPC, L * K1], bf, tag="EE", name="EE")
        EUt = pool.tile([BPC, L * K1], bf, tag="EU", name="EU")
        nc.scalar.dma_start(TBL[:, :], tbl_d[:, :])
        CW = L * K1 // 4
        bounds = [0, CW // 8, CW // 4 + CW // 8, CW, 2 * CW, 3 * CW, 4 * CW]
        for c in range(len(bounds) - 1):
            lo, hi = bounds[c], bounds[c + 1]
            nc.scalar.dma_start(EEt[:, lo:hi], ee_d[:, lo:hi])
            nc.gpsimd.dma_start(EUt[:, lo:hi], eeU_d[:, lo:hi])

        def tb(off, n):
            return TBL[:, off:off + n]

        v = nc.vector

        # ---- DP loop state --------------------------------------------------
        x_pp = [pool.tile([BPC, XW], bf, tag="x_a", name="x_a"),
                pool.tile([BPC, XW], bf, tag="x_b", name="x_b")]
        MUU = pool.tile([BPC, K1], bf, tag="MUU", name="MUU")
        Vb = pool.tile([BPC, K1], bf, tag="Vb", name="Vb")
        GGb = pool.tile([BPC, XW], bf, tag="GGb", name="GGb")
        delta = pool.tile([BPC, K1], bf, tag="delta", name="delta")
        tpad = pool.tile([BPC, K + 3], bf, tag="tpad", name="tpad")
        r12 = pool.tile([BPC, XW], bf, tag="r12", name="r12")
        ya = pool.tile([BPC, K1 + 1], bf, tag="ya", name="ya")

        v.memset(x_pp[1][:, :], 0.0)
        v.memset(tpad[:, :], 1.0)
        v.tensor_copy(Vb[:, :], tb(OFF_V, K1))
        v.tensor_copy(MUU[:, :], tb(OFF_MUU0, K1))
        v.tensor_copy(GGb[:, :], tb(OFF_GG, XW))
        v.tensor_copy(x_pp[0][:, :], tb(OFF_X0, XW))

        def dp_step(l):
            X, Xn = x_pp[l % 2], x_pp[(l + 1) % 2]
            v.tensor_tensor_scan(delta[:, :], Vb[:, :], MUU[:, :], 0.0,
                                 mult, add)
            v.tensor_mul(r12[:, 0:YO + K1], GGb[:, 0:YO + K1],
                         X[:, 0:YO + K1])
            v.tensor_add(tpad[:, 2:K + 2], X[:, YO:YO + K], delta[:, 0:K])
            v.tensor_add(ya[:, 0:K1], r12[:, 0:K1], r12[:, YO:YO + K1])
            v.tensor_mul(Xn[:, 0:K1], EEt[:, l * K1:(l + 1) * K1],
                         tpad[:, 1:K1 + 1])
            v.tensor_mul(MUU[:, 0:K1], EUt[:, l * K1:(l + 1) * K1],
                         tpad[:, 0:K1])
            v.tensor_add(Xn[:, YO:YO + K1], Xn[:, 0:K1], ya[:, 0:K1])

        for l in range(L):
            dp_step(l)

        # readout: ship tf[K] (f32); log/KLD/mean happen on host in f64
        Xf = x_pp[L % 2]
        tf = pool.tile([BPC, K1], f32, tag="tf", name="tf")
        v.tensor_tensor_scan(delta[:, :], Vb[:, :], MUU[:, :], 0.0, mult, add)
        v.tensor_add(tf[:, :], Xf[:, YO:YO + K1], delta[:, :])
        nc.sync.dma_start(out_d[:, :], tf[:, K:K1])

    nc.compile()
    return nc


def _get_nc():
    if "nc" not in _CACHED:
        _CACHED["nc"] = _build_bass()
    return _CACHED["nc"]


def kernel(batch_input, transition_probs, emission_probs, mus, logvars):
    from concourse.bass_utils import run_bass_kernel_spmd

    tbl, ee, eeU = _host_tables(batch_input, transition_probs, emission_probs,
                                mus, logvars)
    nc = _get_nc()
    in_maps = [{"tbl": tbl[c * BPC:(c + 1) * BPC],
                "ee": ee[c * BPC:(c + 1) * BPC],
                "eeU": eeU[c * BPC:(c + 1) * BPC]} for c in range(N_CORES)]
    res = run_bass_kernel_spmd(nc, in_maps, list(range(N_CORES)))
    tfK = np.concatenate([np.asarray(r["loss"])[:, 0] for r in res.results])
    nll = -(np.log(tfK.astype(np.float64)) + (LOGACC0 - L * C_DECAY))
    lv = np.asarray(logvars, np.float64)
    mu = np.asarray(mus, np.float64)
    kld = -0.5 * (1.0 + lv - mu ** 2 - np.exp(lv)).sum(axis=1)
    return np.float32(np.mean(nll + kld))
